# revision 1
# baseline (speedup 1.0000x reference)
"""AttentiveProtoFusion kernel for 8 TRN2 NeuronCores.

Math (equivalent to reference, ~14x fewer FLOPs):
    q  = sent @ Wq + bq                      [n, 768]
    q' = q @ Wk^T                            [n, 768]
    scores[n,p] = sum_c proto[n,p,c] * q'[n,c]   (+ q.bk, constant over p ->
                                                  dropped: softmax invariant)
    w = softmax(scores, axis=p)
    ctx[n,c] = sum_p w[n,p] * proto[n,p,c]

Sharding: pure data-parallel over the 2048 tokens (B*S), 256 tokens/core.
Per core, tokens live on partitions, in 2 blocks of 128. sent and Wk are
staged host-side in transposed layout (pure relayout; same bytes DMA'd)
so the TensorEngine does no transposes at all.

The softmax-weighted pooling runs ONLINE over chunks of CH prototypes
with a fixed exponent frame Mhat = max(chunk0)+60 (statistically safe:
scores are N(0, ||q'||^2) per token; a later score would need a ~4.8
sigma excursion past the chunk-0 max to overflow, and Z >= e^-60 keeps
well clear of denormals; U/Z equals softmax exactly). Proto tiles are
consumed and their SBUF slots recycled as soon as their chunk is done -
no block-wide barrier.

Engine plan:
  PE    : the two small projection matmuls (no transposes).
  DVE   : affine_mul_reduce (custom fused mul+reduce) for most scores;
          fused MAC (scalar_tensor_tensor) on ctx cols [0:A].
  ACT   : exp; per-partition-scale multiplies for ctx cols [A:]; the
          accum-reduce for GPSIMD-computed score products.
  GPSIMD: a slice of the score multiplies + ctx accumulate adds [A:].
  DMA   : streams proto (24.6 MB/core) - the roofline.
"""

import sys

for _p in ("/opt/trn_rl_repo", "/opt/pypackages"):
    if _p not in sys.path:
        sys.path.append(_p)

import numpy as np

B, S, P, D_SENT, D_CTX = 4, 512, 32, 1024, 768
N_CORES = 8
TOK = B * S                    # 2048
TPC = TOK // N_CORES           # 256 tokens per core
BLK = 128                      # tokens per block
NBLK = TPC // BLK              # 2
PG = 2                         # prototypes per DMA tile
NPG = P // PG                  # 16 proto tiles per block
CH = 8                         # prototypes per online chunk
NCH = P // CH                  # 4 chunks per block
TPCH = CH // PG                # tiles per chunk
PPOOL_BUFS = 18

CTX_DV = 768                   # all ctx adds on DVE (GPSIMD add rate is poor)
GPS_SCORE_P = {15, 19}   # scores routed GPS+ACT (not chunk 0 or 3)

_NC = None


def _build():
    import concourse.bass as bass
    import concourse.tile as tile
    from concourse import bacc, mybir

    f32 = mybir.dt.float32
    Alu = mybir.AluOpType
    Act = mybir.ActivationFunctionType
    X = mybir.AxisListType.X

    nc = bacc.Bacc("TRN2", target_bir_lowering=False)

    sentT_d = nc.dram_tensor("sentT", [D_SENT, TPC], f32, kind="ExternalInput")
    proto_d = nc.dram_tensor("proto", [TPC, P, D_CTX], f32, kind="ExternalInput")
    w_d = nc.dram_tensor("w", [D_SENT, D_CTX], f32, kind="ExternalInput")
    bp_d = nc.dram_tensor("bp", [1, D_CTX], f32, kind="ExternalInput")
    out_d = nc.dram_tensor("out", [TPC, D_CTX], f32, kind="ExternalOutput")

    DS = D_SENT // 128   # 8 chunks of the sent feature dim
    DC = D_CTX // 128    # 6 chunks of the ctx feature dim
    EH = D_CTX // 2      # 384

    with tile.TileContext(nc) as tc:
        with (
            tc.tile_pool(name="persist", bufs=1) as persist,
            tc.tile_pool(name="wpool", bufs=1) as wpool,
            tc.tile_pool(name="ppool", bufs=PPOOL_BUFS) as ppool,
            tc.tile_pool(name="small", bufs=4) as small,
            tc.tile_pool(name="scratch", bufs=2) as scratch,
            tc.tile_pool(name="tmpp", bufs=3) as tmpp,
            tc.tile_pool(name="psum", bufs=2, space="PSUM") as psum,
        ):
            qp_sb = persist.tile([128, NBLK, D_CTX], f32)   # q' per block [n, e]
            scores = persist.tile([128, NBLK, P], f32)
            expw = persist.tile([128, NBLK, P], f32)        # exp(s - Mhat)
            U = persist.tile([128, NBLK, D_CTX], f32)       # ctx numerator
            negMhat = persist.tile([128, NBLK, 1], f32)
            clampv = persist.tile([128, NBLK, 1], f32)      # Mhat + 80
            Zrun = persist.tile([128, NBLK, 1], f32)

            # ------------- weights + projection (folded, no transposes) ---
            # qp[n, e] = sum_d sent[n, d] * W[d, e] + bp[e],
            # W = Wq @ Wk^T and bp = bq @ Wk^T folded host-side.
            sentT_sb = wpool.tile([128, DS, TPC], f32)      # sent^T[(dd p), n]
            nc.sync.dma_start(
                out=sentT_sb[:],
                in_=sentT_d[:].rearrange("(dd p) n -> p dd n", p=128),
            )
            w_sb = wpool.tile([128, DS, D_CTX], f32)        # W[(dd p), e]
            nc.sync.dma_start(
                out=w_sb[:], in_=w_d[:].rearrange("(dd p) e -> p dd e", p=128)
            )
            bp_sb = wpool.tile([1, D_CTX], f32)
            nc.sync.dma_start(out=bp_sb[:], in_=bp_d[:])
            ones_sb = wpool.tile([1, 128], f32)
            nc.vector.memset(ones_sb[:], 1.0)

            for b in range(NBLK):
                for h in range(2):
                    pp = psum.tile([128, EH], f32, tag="mm")
                    for dd in range(DS):
                        nc.tensor.matmul(
                            pp[:],
                            sentT_sb[:, dd, b * BLK:(b + 1) * BLK],
                            w_sb[:, dd, h * EH:(h + 1) * EH],
                            start=(dd == 0),
                            stop=False,
                        )
                    nc.tensor.matmul(
                        pp[:],
                        ones_sb[0:1, :],
                        bp_sb[0:1, h * EH:(h + 1) * EH],
                        start=False,
                        stop=True,
                    )
                    nc.scalar.copy(out=qp_sb[:, b, h * EH:(h + 1) * EH], in_=pp[:])

            # ---------------- main loop: online softmax-pooling ----------
            for c in range(NCH):
                for b in range(NBLK):
                    t_tiles = []
                    for t in range(TPCH):
                        g = c * TPCH + t
                        T4 = ppool.tile([128, PG, D_CTX], f32, tag="T")
                        nc.sync.dma_start(
                            out=T4[:],
                            in_=proto_d[
                                b * BLK:(b + 1) * BLK, g * PG:(g + 1) * PG, :
                            ],
                        )
                        t_tiles.append(T4)
                        for j in range(PG):
                            p = g * PG + j
                            if p in GPS_SCORE_P:
                                gs = tmpp.tile([128, D_CTX], f32, tag="gscore")
                                nc.gpsimd.tensor_tensor(
                                    out=gs[:], in0=T4[:, j, :],
                                    in1=qp_sb[:, b, :], op=Alu.mult,
                                )
                                nc.scalar.activation(
                                    out=gs[:], in_=gs[:], func=Act.Copy,
                                    accum_out=scores[:, b, p:p + 1],
                                )
                            else:
                                amr_out = scratch.tile(
                                    [128, D_CTX], f32, tag="amr_out"
                                )
                                nc.vector.affine_mul_reduce(
                                    out=amr_out[:],
                                    accum_out=scores[:, b, p:p + 1],
                                    in0=T4[:, j, :],
                                    in1=qp_sb[:, b, :],
                                    scale=1.0,
                                    bias=0.0,
                                )

                    s_ch = scores[:, b, c * CH:(c + 1) * CH]
                    e_ch = expw[:, b, c * CH:(c + 1) * CH]
                    if c == 0:
                        # fixed frame Mhat = max(chunk0) + 30 (see header)
                        m8 = small.tile([128, 1], f32, tag="m8")
                        nc.vector.tensor_reduce(
                            out=m8[:], in_=s_ch, axis=X, op=Alu.max,
                        )
                        # negMhat = -(max + 60); clampv = max + 140
                        nc.vector.tensor_scalar(
                            negMhat[:, b, :], m8[:], -1.0, -60.0,
                            Alu.mult, Alu.add,
                        )
                        nc.vector.tensor_scalar(
                            clampv[:, b, :], m8[:], 1.0, 140.0,
                            Alu.mult, Alu.add,
                        )
                        nc.scalar.activation(
                            out=e_ch, in_=s_ch, func=Act.Exp,
                            bias=negMhat[:, b, :], scale=1.0,
                        )
                        nc.vector.tensor_reduce(
                            out=Zrun[:, b, :], in_=e_ch, axis=X, op=Alu.add,
                        )
                    else:
                        # guard the fixed frame: s <= Mhat + 80 so exp can
                        # never overflow even for extreme outliers
                        nc.vector.tensor_scalar(
                            s_ch, s_ch, clampv[:, b, :], None, Alu.min,
                        )
                        nc.scalar.activation(
                            out=e_ch, in_=s_ch, func=Act.Exp,
                            bias=negMhat[:, b, :], scale=1.0,
                        )
                        zloc = small.tile([128, 1], f32, tag="zloc")
                        nc.vector.tensor_reduce(
                            out=zloc[:], in_=e_ch, axis=X, op=Alu.add,
                        )
                        nc.vector.tensor_tensor(
                            out=Zrun[:, b, :], in0=Zrun[:, b, :], in1=zloc[:],
                            op=Alu.add,
                        )

                    # MACs: U += e_p * T_p  (ACT multiplies, DVE+GPS add)
                    DV = CTX_DV
                    for t in range(TPCH):
                        T4 = t_tiles[t]
                        for j in range(PG):
                            p = (c * TPCH + t) * PG + j
                            e_p = expw[:, b, p:p + 1]
                            if p == 0:
                                nc.scalar.activation(
                                    out=U[:, b, :], in_=T4[:, j, :],
                                    func=Act.Copy, scale=e_p,
                                )
                            else:
                                gtmp = tmpp.tile([128, D_CTX], f32, tag="gtmp")
                                nc.scalar.activation(
                                    out=gtmp[:], in_=T4[:, j, :],
                                    func=Act.Copy, scale=e_p,
                                )
                                nc.vector.tensor_tensor(
                                    out=U[:, b, 0:DV], in0=gtmp[:, 0:DV],
                                    in1=U[:, b, 0:DV], op=Alu.add,
                                )
                                if DV < D_CTX:
                                    nc.gpsimd.tensor_tensor(
                                        out=U[:, b, DV:], in0=gtmp[:, DV:],
                                        in1=U[:, b, DV:], op=Alu.add,
                                    )

            # -- finalize: ctx = U / Z --
            for b in range(NBLK):
                rinv = small.tile([128, 1], f32, tag="rinv")
                nc.vector.reciprocal(out=rinv[:], in_=Zrun[:, b, :])
                nc.vector.tensor_scalar(
                    U[:, b, 0:384], U[:, b, 0:384], rinv[:], None, Alu.mult,
                )
                nc.scalar.activation(
                    out=U[:, b, 384:], in_=U[:, b, 384:], func=Act.Copy,
                    scale=rinv[:],
                )
                nc.sync.dma_start(
                    out=out_d[b * BLK:(b + 1) * BLK, :], in_=U[:, b, :]
                )

    nc.compile()
    return nc


def _get_nc():
    global _NC
    if _NC is None:
        _NC = _build()
    return _NC


def _make_in_maps(sent_vecs, proto_vecs, Wq, bq, Wk):
    sent = np.asarray(sent_vecs, dtype=np.float32).reshape(TOK, D_SENT)
    sentT = np.ascontiguousarray(sent.T)                      # [D_SENT, TOK]
    proto = np.ascontiguousarray(
        np.asarray(proto_vecs, dtype=np.float32).reshape(TOK, P, D_CTX)
    )
    wq = np.asarray(Wq, dtype=np.float32)
    bq = np.asarray(bq, dtype=np.float32).reshape(1, D_CTX)
    wk = np.asarray(Wk, dtype=np.float32)
    # fold the projection weights host-side: qp = sent @ W + bp
    w = np.ascontiguousarray(wq @ wk.T)
    bp = np.ascontiguousarray(bq @ wk.T)
    in_maps = []
    for i in range(N_CORES):
        sl = slice(i * TPC, (i + 1) * TPC)
        in_maps.append(
            {
                "sentT": np.ascontiguousarray(sentT[:, sl]),
                "proto": np.ascontiguousarray(proto[sl]),
                "w": w,
                "bp": bp,
            }
        )
    return in_maps


def _ensure_ntff_hook():
    """The agent image's antenv lacks axon_hooks; shim it so trace=True
    can capture NTFF profiles via the libaxon ctypes path."""
    try:
        from antenv.axon_hooks import get_axon_ntff_profile_hook  # noqa: F401
        return
    except ImportError:
        pass
    import types

    import antenv
    from trn_agent_boot.trn_boot import _ntff_profile_via_ctypes

    mod = types.ModuleType("antenv.axon_hooks")
    mod._hook = _ntff_profile_via_ctypes("/opt/axon/libaxon_pjrt.so")
    mod.get_axon_ntff_profile_hook = lambda: mod._hook
    mod.set_axon_ntff_profile_hook = lambda h: setattr(mod, "_hook", h)
    sys.modules["antenv.axon_hooks"] = mod
    antenv.axon_hooks = mod


def run(sent_vecs, proto_vecs, Wq, bq, Wk, bk=None, trace=False, **kw):
    """Returns (out[4,512,768] float32, BassKernelResults)."""
    from concourse.bass_utils import run_bass_kernel_spmd

    if trace:
        _ensure_ntff_hook()
    nc = _get_nc()
    in_maps = _make_in_maps(sent_vecs, proto_vecs, Wq, bq, Wk)
    res = run_bass_kernel_spmd(
        nc, in_maps, core_ids=list(range(N_CORES)), trace=trace
    )
    outs = [np.asarray(res.results[i]["out"]) for i in range(N_CORES)]
    full = np.concatenate(outs, axis=0).reshape(B, S, D_CTX).astype(np.float32)
    return full, res


def kernel(sent_vecs, proto_vecs, Wq, bq, Wk, bk=None, **kw):
    out, _ = run(sent_vecs, proto_vecs, Wq, bq, Wk, bk)
    return out


if __name__ == "__main__":
    nc = _get_nc()
    print("build + compile OK")



# revision 3
# speedup vs baseline: 1.6590x; 1.6590x over previous
"""AttentiveProtoFusion kernel for 8 TRN2 NeuronCores.

Math (equivalent to reference, ~14x fewer FLOPs):
    q  = sent @ Wq + bq                      [n, 768]
    q' = q @ Wk^T                            [n, 768]  (W = Wq@Wk^T folded host-side)
    scores[n,p] = sum_c proto[n,p,c] * q'[n,c]   (+ q.bk const over p -> dropped)
    w = softmax(scores, axis=p)
    ctx[n,c] = sum_p w[n,p] * proto[n,p,c]

Sharding: data-parallel over the 2048 tokens (B*S), 256/core, 2 blocks of
128 tokens (tokens on partitions).

Numerics: proto/sent/W are staged host-side in fp16 (rel err ~2e-3 vs the
2e-2 gate; halves the DMA stream to 12 MiB/core). Softmax per block runs
in 2 chunks of 16 prototypes with a chunk-local TRUE max, so exp() lands
in (0,1] - always fp16-safe - and the two chunk accumulators U_a, U_b are
merged at the end with gamma = exp(M_chunk - M_block) scale factors.

Engine plan (per 128-token block):
  PE    : q' projection; the pooling MAC U += e_p * proto_p expressed as
          matmul(lhsT=diag(e_p), rhs=proto_p) accumulating in PSUM fp32.
  DVE   : score passes via fused scalar_tensor_tensor (out=proto*q',
          accum_out=score); builds diag(e_p) = eye * e_p (4x fp16 TS);
          softmax maxes/sums; merge MACs.
  GPSIMD: score multiplies for 5 of 16 protos per chunk (load balance).
  ACT   : accumulates the GPSIMD products; exp; q' PSUM->SBUF copies;
          merge scale-copies.
  DMA   : streams proto fp16 (12 MiB/core) - the roofline.
The emission is software-pipelined: chunk k's diag+MAC are emitted after
chunk k+1's scores so the PE drains while DVE/GPS work the next chunk.
"""

import sys

for _p in ("/opt/trn_rl_repo", "/opt/pypackages"):
    if _p not in sys.path:
        sys.path.append(_p)

import numpy as np

B, S, P, D_SENT, D_CTX = 4, 512, 32, 1024, 768
N_CORES = 8
TOK = B * S                    # 2048
TPC = TOK // N_CORES           # 256 tokens per core
BLK = 128                      # tokens per block
NBLK = TPC // BLK              # 2
HC = 16                        # protos per softmax chunk (2 chunks/block)
PG = 8                         # protos per DMA tile
EH = D_CTX // 2                # 384 = one PSUM-bank-sized half
DS = D_SENT // 128             # 8 contraction chunks for the projection

# per-chunk score routing: proto indices (within the 16-chunk) on GPSIMD
GPS_SET = (0, 1, 2, 3, 4)

_NC = None


def _build():
    import concourse.bass as bass  # noqa: F401
    import concourse.tile as tile
    from concourse import bacc, mybir

    f32 = mybir.dt.float32
    f16 = mybir.dt.float16
    Alu = mybir.AluOpType
    Act = mybir.ActivationFunctionType
    X = mybir.AxisListType.X

    nc = bacc.Bacc("TRN2", target_bir_lowering=False)

    sentT_d = nc.dram_tensor("sentT", [D_SENT, TPC], f16, kind="ExternalInput")
    proto_d = nc.dram_tensor("proto", [TPC, P, D_CTX], f16, kind="ExternalInput")
    w_d = nc.dram_tensor("w", [D_SENT, D_CTX], f16, kind="ExternalInput")
    bp_d = nc.dram_tensor("bp", [1, D_CTX], f16, kind="ExternalInput")
    eye_d = nc.dram_tensor("eye", [128, 128], f16, kind="ExternalInput")
    out_d = nc.dram_tensor("out", [TPC, D_CTX], f32, kind="ExternalOutput")

    with tile.TileContext(nc) as tc:
        with (
            tc.tile_pool(name="persist", bufs=1) as persist,
            tc.tile_pool(name="wpool", bufs=1) as wpool,
            tc.tile_pool(name="ppool", bufs=6) as ppool,
            tc.tile_pool(name="dpool", bufs=4) as dpool,
            tc.tile_pool(name="junk", bufs=2) as junkp,
            tc.tile_pool(name="gsp", bufs=3) as gsp,
            tc.tile_pool(name="small", bufs=8) as small,
            tc.tile_pool(name="psum", bufs=8, space="PSUM") as psum,
        ):
            scores = persist.tile([128, NBLK, P], f32)
            expw = persist.tile([128, NBLK, P], f32)
            Mc = persist.tile([128, NBLK, 2], f32)      # chunk maxes
            negMc = persist.tile([128, NBLK, 2], f32)
            Zc = persist.tile([128, NBLK, 2], f32)      # chunk exp-sums
            qp_sb = persist.tile([128, NBLK, D_CTX], f32)
            out_sb = persist.tile([128, NBLK, D_CTX], f32)

            # ---------------- weights + projection --------------------
            sentT_sb = wpool.tile([128, DS, TPC], f16)
            nc.sync.dma_start(
                out=sentT_sb[:],
                in_=sentT_d[:].rearrange("(dd p) n -> p dd n", p=128),
            )
            w_sb = wpool.tile([128, DS, D_CTX], f16)
            nc.sync.dma_start(
                out=w_sb[:], in_=w_d[:].rearrange("(dd p) e -> p dd e", p=128)
            )
            bp_sb = wpool.tile([1, D_CTX], f16)
            nc.sync.dma_start(out=bp_sb[:], in_=bp_d[:])
            eye_sb = wpool.tile([128, 128], f16)
            nc.sync.dma_start(out=eye_sb[:], in_=eye_d[:])
            ones_sb = wpool.tile([1, 128], f16)
            nc.vector.memset(ones_sb[:], 1.0)

            # q' for both blocks up front (PSUM bufs recycle into U tiles)
            for b in range(NBLK):
                for h in range(2):
                    pp = psum.tile([128, EH], f32, tag="ps")
                    for dd in range(DS):
                        nc.tensor.matmul(
                            pp[:],
                            sentT_sb[:, dd, b * BLK:(b + 1) * BLK],
                            w_sb[:, dd, h * EH:(h + 1) * EH],
                            start=(dd == 0),
                            stop=False,
                        )
                    nc.tensor.matmul(
                        pp[:],
                        ones_sb[0:1, :],
                        bp_sb[0:1, h * EH:(h + 1) * EH],
                        start=False,
                        stop=True,
                    )
                    nc.scalar.copy(
                        out=qp_sb[:, b, h * EH:(h + 1) * EH], in_=pp[:]
                    )

            # ---------------- chunked softmax-pooling -----------------
            chunks = [(b, c) for b in range(NBLK) for c in range(2)]
            # U accumulators per chunk: (lo, hi) psum tiles
            U = {}
            tiles = {}

            def emit_scores(k):
                b, c = chunks[k]
                p0 = c * HC
                t_list = []
                for t in range(HC // PG):
                    T = ppool.tile([128, PG, D_CTX], f16, tag="T")
                    nc.sync.dma_start(
                        out=T[:],
                        in_=proto_d[
                            b * BLK:(b + 1) * BLK,
                            p0 + t * PG:p0 + (t + 1) * PG, :,
                        ],
                    )
                    t_list.append(T)
                tiles[k] = t_list
                for j in range(HC):
                    p = p0 + j
                    T = t_list[j // PG]
                    src = T[:, j % PG, :]
                    if j in GPS_SET:
                        gs = gsp.tile([128, D_CTX], f16, tag="gs")
                        nc.gpsimd.tensor_tensor(
                            out=gs[:], in0=src, in1=qp_sb[:, b, :],
                            op=Alu.mult,
                        )
                        jk = junkp.tile([128, D_CTX], f16, tag="junk")
                        nc.scalar.activation(
                            out=jk[:], in_=gs[:], func=Act.Copy,
                            accum_out=scores[:, b, p:p + 1],
                        )
                    else:
                        jk = junkp.tile([128, D_CTX], f16, tag="junk")
                        nc.vector.scalar_tensor_tensor(
                            out=jk[:],
                            in0=src,
                            scalar=0.0,
                            in1=qp_sb[:, b, :],
                            op0=Alu.bypass,
                            op1=Alu.mult,
                            accum_out=scores[:, b, p:p + 1],
                        )
                # chunk max -> exp
                s_ch = scores[:, b, p0:p0 + HC]
                nc.vector.tensor_reduce(
                    out=Mc[:, b, c:c + 1], in_=s_ch, axis=X, op=Alu.max,
                )
                nc.vector.tensor_scalar(
                    negMc[:, b, c:c + 1], Mc[:, b, c:c + 1],
                    -1.0, None, Alu.mult,
                )
                nc.scalar.activation(
                    out=expw[:, b, p0:p0 + HC], in_=s_ch, func=Act.Exp,
                    bias=negMc[:, b, c:c + 1], scale=1.0,
                )

            def emit_mac(k):
                b, c = chunks[k]
                p0 = c * HC
                ulo = psum.tile([128, EH], f32, tag="ps")
                uhi = psum.tile([128, EH], f32, tag="ps")
                U[k] = (ulo, uhi)
                for j in range(HC):
                    p = p0 + j
                    T = tiles[k][j // PG]
                    dg = dpool.tile([128, 128], f16, tag="dg")
                    nc.vector.tensor_scalar(
                        dg[:], eye_sb[:], expw[:, b, p:p + 1], None, Alu.mult,
                    )
                    src = T[:, j % PG, :]
                    nc.tensor.matmul(
                        ulo[:], dg[:], src[:, 0:EH],
                        start=(j == 0), stop=(j == HC - 1),
                    )
                    nc.tensor.matmul(
                        uhi[:], dg[:], src[:, EH:],
                        start=(j == 0), stop=(j == HC - 1),
                    )
                nc.vector.tensor_reduce(
                    out=Zc[:, b, c:c + 1], in_=expw[:, b, p0:p0 + HC],
                    axis=X, op=Alu.add,
                )

            def emit_merge(b):
                ka, kb = 2 * b, 2 * b + 1
                mg = small.tile([128, 1], f32, tag="mg")
                nc.vector.tensor_tensor(
                    out=mg[:], in0=Mc[:, b, 0:1], in1=Mc[:, b, 1:2],
                    op=Alu.max,
                )
                nmg = small.tile([128, 1], f32, tag="nmg")
                nc.vector.tensor_scalar(nmg[:], mg[:], -1.0, None, Alu.mult)
                ga = small.tile([128, 1], f32, tag="ga")
                nc.scalar.activation(
                    out=ga[:], in_=Mc[:, b, 0:1], func=Act.Exp,
                    bias=nmg[:], scale=1.0,
                )
                gb = small.tile([128, 1], f32, tag="gb")
                nc.scalar.activation(
                    out=gb[:], in_=Mc[:, b, 1:2], func=Act.Exp,
                    bias=nmg[:], scale=1.0,
                )
                za = small.tile([128, 1], f32, tag="za")
                nc.vector.tensor_tensor(
                    out=za[:], in0=ga[:], in1=Zc[:, b, 0:1], op=Alu.mult,
                )
                zb = small.tile([128, 1], f32, tag="zb")
                nc.vector.tensor_tensor(
                    out=zb[:], in0=gb[:], in1=Zc[:, b, 1:2], op=Alu.mult,
                )
                zt = small.tile([128, 1], f32, tag="zt")
                nc.vector.tensor_tensor(
                    out=zt[:], in0=za[:], in1=zb[:], op=Alu.add,
                )
                rinv = small.tile([128, 1], f32, tag="rinv")
                nc.vector.reciprocal(out=rinv[:], in_=zt[:])
                gap = small.tile([128, 1], f32, tag="gap")
                nc.vector.tensor_tensor(
                    out=gap[:], in0=ga[:], in1=rinv[:], op=Alu.mult,
                )
                gbp = small.tile([128, 1], f32, tag="gbp")
                nc.vector.tensor_tensor(
                    out=gbp[:], in0=gb[:], in1=rinv[:], op=Alu.mult,
                )
                for h in range(2):
                    sl = slice(h * EH, (h + 1) * EH)
                    nc.scalar.activation(
                        out=out_sb[:, b, sl], in_=U[ka][h][:],
                        func=Act.Copy, scale=gap[:],
                    )
                    nc.vector.scalar_tensor_tensor(
                        out=out_sb[:, b, sl],
                        in0=U[kb][h][:],
                        scalar=gbp[:],
                        in1=out_sb[:, b, sl],
                        op0=Alu.mult,
                        op1=Alu.add,
                    )
                nc.sync.dma_start(
                    out=out_d[b * BLK:(b + 1) * BLK, :], in_=out_sb[:, b, :]
                )

            for k in range(len(chunks)):
                emit_scores(k)
                if k > 0:
                    emit_mac(k - 1)
            emit_mac(len(chunks) - 1)
            emit_merge(0)
            emit_merge(1)

    nc.compile()
    return nc


def _get_nc():
    global _NC
    if _NC is None:
        _NC = _build()
    return _NC


def _make_in_maps(sent_vecs, proto_vecs, Wq, bq, Wk):
    f16 = np.float16
    sent = np.asarray(sent_vecs, dtype=np.float32).reshape(TOK, D_SENT)
    sentT = np.ascontiguousarray(sent.T.astype(f16))          # [D_SENT, TOK]
    proto = np.asarray(proto_vecs, dtype=np.float32).reshape(TOK, P, D_CTX)
    proto16 = np.ascontiguousarray(proto.astype(f16))
    wq = np.asarray(Wq, dtype=np.float32)
    bq = np.asarray(bq, dtype=np.float32).reshape(1, D_CTX)
    wk = np.asarray(Wk, dtype=np.float32)
    w = np.ascontiguousarray((wq @ wk.T).astype(f16))
    bp = np.ascontiguousarray((bq @ wk.T).astype(f16))
    eye = np.ascontiguousarray(np.eye(128, dtype=f16))
    in_maps = []
    for i in range(N_CORES):
        sl = slice(i * TPC, (i + 1) * TPC)
        in_maps.append(
            {
                "sentT": np.ascontiguousarray(sentT[:, sl]),
                "proto": np.ascontiguousarray(proto16[sl]),
                "w": w,
                "bp": bp,
                "eye": eye,
            }
        )
    return in_maps


def _ensure_ntff_hook():
    """The agent image's antenv lacks axon_hooks; shim it so trace=True
    can capture NTFF profiles via the libaxon ctypes path."""
    try:
        from antenv.axon_hooks import get_axon_ntff_profile_hook  # noqa: F401
        return
    except ImportError:
        pass
    import types

    import antenv
    from trn_agent_boot.trn_boot import _ntff_profile_via_ctypes

    mod = types.ModuleType("antenv.axon_hooks")
    mod._hook = _ntff_profile_via_ctypes("/opt/axon/libaxon_pjrt.so")
    mod.get_axon_ntff_profile_hook = lambda: mod._hook
    mod.set_axon_ntff_profile_hook = lambda h: setattr(mod, "_hook", h)
    sys.modules["antenv.axon_hooks"] = mod
    antenv.axon_hooks = mod


def run(sent_vecs, proto_vecs, Wq, bq, Wk, bk=None, trace=False, **kw):
    """Returns (out[4,512,768] float32, BassKernelResults)."""
    from concourse.bass_utils import run_bass_kernel_spmd

    if trace:
        _ensure_ntff_hook()
    nc = _get_nc()
    in_maps = _make_in_maps(sent_vecs, proto_vecs, Wq, bq, Wk)
    res = run_bass_kernel_spmd(
        nc, in_maps, core_ids=list(range(N_CORES)), trace=trace
    )
    outs = [np.asarray(res.results[i]["out"]) for i in range(N_CORES)]
    full = np.concatenate(outs, axis=0).reshape(B, S, D_CTX).astype(np.float32)
    return full, res


def kernel(sent_vecs, proto_vecs, Wq, bq, Wk, bk=None, **kw):
    out, _ = run(sent_vecs, proto_vecs, Wq, bq, Wk, bk)
    return out


if __name__ == "__main__":
    nc = _get_nc()
    print("build + compile OK")


# revision 8
# speedup vs baseline: 1.6632x; 1.0025x over previous
"""AttentiveProtoFusion kernel for 8 TRN2 NeuronCores.

Math (equivalent to reference, ~14x fewer FLOPs):
    q' = sent @ (Wq @ Wk^T) + bq @ Wk^T      [n, 768]   (folded host-side)
    scores[n,p] = sum_c proto[n,p,c] * q'[n,c]
    w = softmax(scores, axis=p);  ctx[n,c] = sum_p w[n,p] * proto[n,p,c]

Sharding: data-parallel over the 2048 tokens (B*S), 256/core, 2 blocks of
128 tokens (tokens on partitions). proto/sent/W staged host-side in fp16
(rel err ~2.8e-3 vs the 2e-2 gate; halves DMA to 12 MiB/core).

Softmax uses the fixed exponent frame proven in the fp32 baseline:
Mhat = max(chunk0) + 60, scores clamped at Mhat + 80, so e = exp(s - Mhat)
spans up to e^80 = 5.5e34 - safely inside bf16 range. The pooling weights
e are materialised as bf16 DIAGONAL matrices and the whole MAC
U += e_p * proto_p runs on the TensorEngine as
matmul(lhsT=diag(e_p) bf16, rhs=proto_p fp16) accumulating in PSUM fp32
(mixed 16-bit dtypes verified exact on HW; ldweights pipelines behind the
previous matmul, 163 ns/384-col matmul at full clock).

Engine plan (measured costs):
  DVE   : score passes via fused scalar_tensor_tensor+accum (~1.1 us);
          some diag builds diag=TS(eye*e_p) (~345 ns); softmax sm/frames.
  GPSIMD: score products for ~13 of 32 protos per block (TT ~1.65 us).
  ACT   : accumulates GPSIMD products (~1.3 us); per-chunk exp for Z;
          other diag builds via dg=Exp(eyeNEG + sm_p) bf16 (~385 ns,
          eyeNEG has -60000 off-diagonal so exp -> 0); q' copies; final
          ctx = U * (1/Z) PSUM->SBUF copies.
  PE    : q' projection + the 128 MAC matmuls/block.
  DMA   : proto fp16 stream, 12 MiB/core.
Emission is software-pipelined one chunk deep (DVE diags and the sm of
GPS-carrying chunks are emitted after the next chunk's scores) so no
engine waits at a chunk barrier.
"""

import sys

for _p in ("/opt/trn_rl_repo", "/opt/pypackages"):
    if _p not in sys.path:
        sys.path.append(_p)

import numpy as np

B, S, P, D_SENT, D_CTX = 4, 512, 32, 1024, 768
N_CORES = 8
TOK = B * S                    # 2048
TPC = TOK // N_CORES           # 256 tokens per core
BLK = 128                      # tokens per block
NBLK = TPC // BLK              # 2
CH = 8                         # protos per chunk
NCH = P // CH                  # 4 chunks per block
EH = D_CTX // 2                # 384 = PSUM-bank-sized half
DS = D_SENT // 128             # 8 contraction chunks for the projection

# tuning knobs: per chunk index, how many protos go to GPSIMD (from the
# front of the chunk) and which chunk offsets build their diag on ACT.
GPS_N = {0: 0, 1: 5, 2: 4, 3: 4}
DIAG_ACT = {0: (1, 3, 5), 1: (1, 3, 5), 2: (1, 3, 5),
            3: (0, 1, 2, 3, 4, 5, 6, 7)}

_NC = None


def _build():
    import concourse.tile as tile
    from concourse import bacc, mybir

    f32 = mybir.dt.float32
    f16 = mybir.dt.float16
    bf16 = mybir.dt.bfloat16
    Alu = mybir.AluOpType
    Act = mybir.ActivationFunctionType
    X = mybir.AxisListType.X

    nc = bacc.Bacc("TRN2", target_bir_lowering=False)

    sentT_d = nc.dram_tensor("sentT", [D_SENT, TPC], f16, kind="ExternalInput")
    proto_d = nc.dram_tensor("proto", [TPC, P, D_CTX], f16, kind="ExternalInput")
    w_d = nc.dram_tensor("w", [D_SENT, D_CTX], f16, kind="ExternalInput")
    bp_d = nc.dram_tensor("bp", [1, D_CTX], f16, kind="ExternalInput")
    eye_d = nc.dram_tensor("eye", [128, 128], bf16, kind="ExternalInput")
    eyeneg_d = nc.dram_tensor("eyeneg", [128, 128], f32, kind="ExternalInput")
    out_d = nc.dram_tensor("out", [TPC, D_CTX], f32, kind="ExternalOutput")

    with tile.TileContext(nc) as tc:
        with (
            tc.tile_pool(name="persist", bufs=1) as persist,
            tc.tile_pool(name="wpool", bufs=1) as wpool,
            tc.tile_pool(name="ppool", bufs=8) as ppool,
            tc.tile_pool(name="dpool", bufs=10) as dpool,
            tc.tile_pool(name="junk", bufs=2) as junkp,
            tc.tile_pool(name="gsp", bufs=3) as gsp,
            tc.tile_pool(name="small", bufs=6) as small,
            tc.tile_pool(name="psum", bufs=8, space="PSUM") as psum,
        ):
            scores = persist.tile([128, NBLK, P], f32)
            sm = persist.tile([128, NBLK, P], f32)       # clamped, shifted
            expw = persist.tile([128, NBLK, P], f32)
            negM = persist.tile([128, NBLK, 1], f32)
            clampv = persist.tile([128, NBLK, 1], f32)
            qp_sb = persist.tile([128, NBLK, D_CTX], f16)
            out_sb = persist.tile([128, NBLK, D_CTX], f32)

            # ---------------- weights + projection --------------------
            w_sb = wpool.tile([128, DS, D_CTX], f16)
            nc.sync.dma_start(
                out=w_sb[:], in_=w_d[:].rearrange("(dd p) e -> p dd e", p=128)
            )
            sentT_sb = wpool.tile([128, DS, TPC], f16)
            nc.sync.dma_start(
                out=sentT_sb[:],
                in_=sentT_d[:].rearrange("(dd p) n -> p dd n", p=128),
            )
            bp_sb = wpool.tile([1, D_CTX], f16)
            nc.sync.dma_start(out=bp_sb[:], in_=bp_d[:])
            eye_sb = wpool.tile([128, 128], bf16)
            nc.sync.dma_start(out=eye_sb[:], in_=eye_d[:])
            eyeneg_sb = wpool.tile([128, 128], f32)
            nc.sync.dma_start(out=eyeneg_sb[:], in_=eyeneg_d[:])
            ones_sb = wpool.tile([1, 128], f16)
            nc.vector.memset(ones_sb[:], 1.0)

            for b in range(NBLK):
                for h in range(2):
                    pp = psum.tile([128, EH], f32, tag="ps")
                    for dd in range(DS):
                        nc.tensor.matmul(
                            pp[:],
                            sentT_sb[:, dd, b * BLK:(b + 1) * BLK],
                            w_sb[:, dd, h * EH:(h + 1) * EH],
                            start=(dd == 0),
                            stop=False,
                        )
                    nc.tensor.matmul(
                        pp[:],
                        ones_sb[0:1, :],
                        bp_sb[0:1, h * EH:(h + 1) * EH],
                        start=False,
                        stop=True,
                    )
                    nc.scalar.copy(
                        out=qp_sb[:, b, h * EH:(h + 1) * EH], in_=pp[:]
                    )

            # ---------------- online softmax-pooling ------------------
            ks = [(b, c) for b in range(NBLK) for c in range(NCH)]
            tiles = {}
            Upsum = {}

            gs_tiles = {}

            def emit_products(k):
                """DMA the chunk tile; GPSIMD products; DVE stt scores.
                ACT accums for the GPSIMD protos are emitted separately
                (emit_accums) so ready diag work can precede them in the
                ACT program."""
                b, c = ks[k]
                p0 = c * CH
                T = ppool.tile([128, CH, D_CTX], f16, tag="T")
                nc.sync.dma_start(
                    out=T[:],
                    in_=proto_d[b * BLK:(b + 1) * BLK, p0:p0 + CH, :],
                )
                tiles[k] = T
                ng = GPS_N[c]
                for j in range(ng):
                    gs = gsp.tile([128, D_CTX], f16, tag="gs")
                    nc.gpsimd.tensor_tensor(
                        out=gs[:], in0=T[:, j, :], in1=qp_sb[:, b, :],
                        op=Alu.mult,
                    )
                    gs_tiles[(k, j)] = gs
                for j in range(ng, CH):
                    p = p0 + j
                    jk = junkp.tile([128, D_CTX], f16, tag="junk")
                    nc.vector.scalar_tensor_tensor(
                        out=jk[:],
                        in0=T[:, j, :],
                        scalar=0.0,
                        in1=qp_sb[:, b, :],
                        op0=Alu.bypass,
                        op1=Alu.mult,
                        accum_out=scores[:, b, p:p + 1],
                    )
                if c == 0:
                    m8 = small.tile([128, 1], f32, tag="m8")
                    nc.vector.tensor_reduce(
                        out=m8[:], in_=scores[:, b, 0:CH], axis=X, op=Alu.max,
                    )
                    nc.vector.tensor_scalar(
                        negM[:, b, :], m8[:], -1.0, -60.0, Alu.mult, Alu.add,
                    )
                    nc.vector.tensor_scalar(
                        clampv[:, b, :], m8[:], 1.0, 140.0, Alu.mult, Alu.add,
                    )

            def emit_accums(k):
                b, c = ks[k]
                p0 = c * CH
                for j in range(GPS_N[c]):
                    p = p0 + j
                    jk = junkp.tile([128, D_CTX], f16, tag="junk")
                    nc.scalar.activation(
                        out=jk[:], in_=gs_tiles.pop((k, j)), func=Act.Copy,
                        accum_out=scores[:, b, p:p + 1],
                    )

            def emit_sm(k):
                # sm = min(s, clamp) + negM, then expw for Z + ACT diags
                b, c = ks[k]
                p0 = c * CH
                nc.vector.tensor_scalar(
                    sm[:, b, p0:p0 + CH], scores[:, b, p0:p0 + CH],
                    clampv[:, b, :], negM[:, b, :], Alu.min, Alu.add,
                )
                nc.scalar.activation(
                    out=expw[:, b, p0:p0 + CH], in_=sm[:, b, p0:p0 + CH],
                    func=Act.Exp, bias=0.0, scale=1.0,
                )

            dgs = {}

            def emit_diag_act(k):
                b, c = ks[k]
                p0 = c * CH
                for j in DIAG_ACT[c]:
                    dg = dpool.tile([128, 128], bf16, tag="dg")
                    nc.scalar.activation(
                        out=dg[:], in_=eyeneg_sb[:], func=Act.Exp,
                        bias=sm[:, b, p0 + j:p0 + j + 1], scale=1.0,
                    )
                    dgs[(k, j)] = dg

            def emit_diag_dve(k):
                b, c = ks[k]
                p0 = c * CH
                for j in range(CH):
                    if j in DIAG_ACT[c]:
                        continue
                    dg = dpool.tile([128, 128], bf16, tag="dg")
                    nc.vector.tensor_scalar(
                        dg[:], eye_sb[:], expw[:, b, p0 + j:p0 + j + 1],
                        None, Alu.mult,
                    )
                    dgs[(k, j)] = dg

            def emit_mac(k):
                b, c = ks[k]
                if c == 0:
                    ulo = psum.tile([128, EH], f32, tag="ps")
                    uhi = psum.tile([128, EH], f32, tag="ps")
                    Upsum[b] = (ulo, uhi)
                ulo, uhi = Upsum[b]
                T = tiles[k]
                order = list(DIAG_ACT[c]) + [
                    j for j in range(CH) if j not in DIAG_ACT[c]
                ]
                for i, j in enumerate(order):
                    dg = dgs.pop((k, j))
                    first = (c == 0 and i == 0)
                    last = (c == NCH - 1 and i == CH - 1)
                    nc.tensor.matmul(
                        ulo[:], dg[:], T[:, j, 0:EH],
                        start=first, stop=last,
                    )
                    nc.tensor.matmul(
                        uhi[:], dg[:], T[:, j, EH:],
                        start=first, stop=last,
                    )

            def emit_final(b):
                z = small.tile([128, 1], f32, tag="z")
                nc.vector.tensor_reduce(
                    out=z[:], in_=expw[:, b, :], axis=X, op=Alu.add,
                )
                rinv = small.tile([128, 1], f32, tag="rinv")
                nc.vector.reciprocal(out=rinv[:], in_=z[:])
                ulo, uhi = Upsum[b]
                nc.scalar.activation(
                    out=out_sb[:, b, 0:EH], in_=ulo[:], func=Act.Copy,
                    scale=rinv[:],
                )
                nc.scalar.activation(
                    out=out_sb[:, b, EH:], in_=uhi[:], func=Act.Copy,
                    scale=rinv[:],
                )
                nc.sync.dma_start(
                    out=out_d[b * BLK:(b + 1) * BLK, :], in_=out_sb[:, b, :]
                )

            # one-chunk-deep software pipeline; per iteration k:
            #   1. products+stt scores for chunk k (GPS + DVE)
            #   2. sm(k-1) on DVE (its ACT accums are well underway)
            #   3. ACT diags + exp for chunk k-1 (ready work first in the
            #      ACT program), then the ACT accums for chunk k
            #   4. DVE diags for k-1, then MAC(k-1) on the PE
            # c==0 chunks have no GPSIMD protos so their sm/ACT-diags
            # happen immediately (the frame comes from their scores).
            def post_scores(k):
                emit_sm(k)
                emit_diag_act(k)

            for k in range(len(ks)):
                b, c = ks[k]
                emit_products(k)
                prev = k - 1
                if prev >= 0:
                    if GPS_N[ks[prev][1]] != 0:
                        post_scores(prev)
                    if GPS_N[c] != 0:
                        emit_accums(k)
                    emit_diag_dve(prev)
                    emit_mac(prev)
                    if ks[prev][1] == NCH - 1:
                        emit_final(ks[prev][0])
                if c == 0:
                    post_scores(k)
            last = len(ks) - 1
            if GPS_N[ks[last][1]] != 0:
                post_scores(last)
            emit_diag_dve(last)
            emit_mac(last)
            emit_final(ks[last][0])

    nc.compile()
    return nc


def _get_nc():
    global _NC
    if _NC is None:
        _NC = _build()
    return _NC


def _make_in_maps(sent_vecs, proto_vecs, Wq, bq, Wk):
    f16 = np.float16
    import ml_dtypes

    sent = np.asarray(sent_vecs, dtype=np.float32).reshape(TOK, D_SENT)
    sentT = np.ascontiguousarray(sent.T.astype(f16))          # [D_SENT, TOK]
    proto = np.asarray(proto_vecs, dtype=np.float32).reshape(TOK, P, D_CTX)
    proto16 = np.ascontiguousarray(proto.astype(f16))
    wq = np.asarray(Wq, dtype=np.float32)
    bq = np.asarray(bq, dtype=np.float32).reshape(1, D_CTX)
    wk = np.asarray(Wk, dtype=np.float32)
    w = np.ascontiguousarray((wq @ wk.T).astype(f16))
    bp = np.ascontiguousarray((bq @ wk.T).astype(f16))
    eye = np.ascontiguousarray(np.eye(128, dtype=ml_dtypes.bfloat16))
    eyeneg = np.ascontiguousarray(
        np.where(np.eye(128, dtype=bool), 0.0, -60000.0).astype(np.float32)
    )
    in_maps = []
    for i in range(N_CORES):
        sl = slice(i * TPC, (i + 1) * TPC)
        in_maps.append(
            {
                "sentT": np.ascontiguousarray(sentT[:, sl]),
                "proto": np.ascontiguousarray(proto16[sl]),
                "w": w,
                "bp": bp,
                "eye": eye,
                "eyeneg": eyeneg,
            }
        )
    return in_maps


def _ensure_ntff_hook():
    """The agent image's antenv lacks axon_hooks; shim it so trace=True
    can capture NTFF profiles via the libaxon ctypes path."""
    try:
        from antenv.axon_hooks import get_axon_ntff_profile_hook  # noqa: F401
        return
    except ImportError:
        pass
    import types

    import antenv
    from trn_agent_boot.trn_boot import _ntff_profile_via_ctypes

    mod = types.ModuleType("antenv.axon_hooks")
    mod._hook = _ntff_profile_via_ctypes("/opt/axon/libaxon_pjrt.so")
    mod.get_axon_ntff_profile_hook = lambda: mod._hook
    mod.set_axon_ntff_profile_hook = lambda h: setattr(mod, "_hook", h)
    sys.modules["antenv.axon_hooks"] = mod
    antenv.axon_hooks = mod


def run(sent_vecs, proto_vecs, Wq, bq, Wk, bk=None, trace=False, **kw):
    """Returns (out[4,512,768] float32, BassKernelResults)."""
    from concourse.bass_utils import run_bass_kernel_spmd

    if trace:
        _ensure_ntff_hook()
    nc = _get_nc()
    in_maps = _make_in_maps(sent_vecs, proto_vecs, Wq, bq, Wk)
    res = run_bass_kernel_spmd(
        nc, in_maps, core_ids=list(range(N_CORES)), trace=trace
    )
    outs = [np.asarray(res.results[i]["out"]) for i in range(N_CORES)]
    full = np.concatenate(outs, axis=0).reshape(B, S, D_CTX).astype(np.float32)
    return full, res


def kernel(sent_vecs, proto_vecs, Wq, bq, Wk, bk=None, **kw):
    out, _ = run(sent_vecs, proto_vecs, Wq, bq, Wk, bk)
    return out


if __name__ == "__main__":
    nc = _get_nc()
    print("build + compile OK")
